# revision 10
# baseline (speedup 1.0000x reference)
"""Content-based addressing read (DNC-style) for Trainium2.

Computes softmax_n( strengths[r] * cos_sim(memory[b,n,:], read_vectors[b,:,r]) )
for B=16, N=32768, W=128, R=8, sharded batch-parallel across 8 NeuronCores
(2 batches per core).

v4: 16-bit datapath + chase-proofed scheduling.
  - gpsimd casting DMA (f32 HBM -> f16 SBUF): HBM traffic unchanged, all
    downstream engines see 16-bit. fp32-path matmuls cost ~200ns fixed each;
    f16 pairs stream at 27-107ns/tile.
  - PE: 8 transposes per 1024-wide f16 PSUM tile, then 8 sim matmuls
    (f16 stationary memT chunk, f16 rv' moving, f32 PSUM accumulate).
  - norms: square (gpsimd/ACT rotation) -> two fold-halves adds (DVE f16 2x)
    -> w-reduce (DVE, f32 out). The fold/reduce for group g are EMITTED two
    groups late: slow producers (7us gpsimd squares) would otherwise be
    chased element-wise by the in-order DVE queue, inheriting their latency.
  - memT PSUM->SBUF drains rotate v/v/s; scps drains on gpsimd.
Softmax math stays fp32; no max subtraction (|scores| <= ~1); the reference's
+1e-8 is a provable fp32 no-op (normalizer ~128); 1/sqrt via exp(-0.5*ln).

Output is stored in DRAM as (b, p, t, r) with n = g*4096 + p*32 + t; the host
re-transposes the 16MB result to (b, n, r).
"""

import sys

for _p in ("/opt/trn_rl_repo",):
    if _p not in sys.path:
        sys.path.insert(0, _p)

from contextlib import ExitStack

import numpy as np

import concourse.bass as bass
import concourse.bacc as bacc
import concourse.tile as tile
from concourse import mybir
from concourse import bass_isa
from concourse.bass_utils import run_bass_kernel_spmd

F32 = mybir.dt.float32
F16 = mybir.dt.float16
AF = mybir.ActivationFunctionType

B, N, W, R = 16, 32768, 128, 8
NCORES = 8
BLOC = B // NCORES          # batches per core
T = N // 128                # 256 n-tiles of 128 per batch
NG = 8                      # DMA groups per batch
TPG = T // NG               # 32 tiles per group (4096 n, 2MB)
CH = 8                      # tiles per PSUM transpose chunk (1024 cols)
NCH = TPG // CH             # chunks per group

# ---- tuning knobs ----
SQUARE_ENGINES = "gv"       # g=GpSimd, s=ScalarE, v=DVE
MEMT_DRAIN = "sssv"          # rotation for memT PSUM->SBUF drains
NORM_DEFER = 3              # groups of deferral for fold/reduce emission
IN_BUFS = 9


def build_program():
    nc = bacc.Bacc("TRN2", target_bir_lowering=False, debug=False, num_devices=NCORES)

    mem = nc.dram_tensor("memory", [BLOC, N, W], F32, kind="ExternalInput").ap()
    rv = nc.dram_tensor("read_vectors", [BLOC, W, R], F32, kind="ExternalInput").ap()
    rs = nc.dram_tensor("read_strengths", [BLOC, R], F32, kind="ExternalInput").ap()
    ident = nc.dram_tensor("identity", [128, 128], F32, kind="ExternalInput").ap()
    ones = nc.dram_tensor("ones", [128, 128], F32, kind="ExternalInput").ap()
    out = nc.dram_tensor("out", [BLOC, 128, T, R], F32, kind="ExternalOutput").ap()

    with ExitStack() as ctx:
        tc = ctx.enter_context(tile.TileContext(nc))

        const_pool = ctx.enter_context(tc.tile_pool(name="const", bufs=1))
        id_t = const_pool.tile([128, 128], F32)
        nc.sync.dma_start(id_t[:], ident)
        ones_t = const_pool.tile([128, 128], F32)
        nc.sync.dma_start(ones_t[:], ones)
        id_h = const_pool.tile([128, 128], F16)
        nc.scalar.copy(id_h[:], id_t[:])
        ones_h = const_pool.tile([128, 128], F16)
        nc.scalar.copy(ones_h[:], ones_t[:])

        in_pool = ctx.enter_context(tc.tile_pool(name="mem_in", bufs=IN_BUFS))
        sq_pool = ctx.enter_context(tc.tile_pool(name="sq", bufs=NORM_DEFER + 2))
        fd_pool = ctx.enter_context(tc.tile_pool(name="fd", bufs=2))
        fd2_pool = ctx.enter_context(tc.tile_pool(name="fd2", bufs=2))
        mtps_pool = ctx.enter_context(tc.tile_pool(name="mtps", bufs=4, space="PSUM"))
        mt_pool = ctx.enter_context(tc.tile_pool(name="mt", bufs=6))
        scps_pool = ctx.enter_context(tc.tile_pool(name="scps", bufs=2, space="PSUM"))
        rtps_pool = ctx.enter_context(tc.tile_pool(name="rtps", bufs=1, space="PSUM"))
        smalls = ctx.enter_context(tc.tile_pool(name="smalls", bufs=2))
        score_pool = ctx.enter_context(tc.tile_pool(name="scores", bufs=2))
        ss_pool = ctx.enter_context(tc.tile_pool(name="ss", bufs=2))

        drain_i = 0
        sq_i = 0

        def emit_norm(ss, sq_g, g, scores, s1, first):
            """fold twice + reduce (DVE), then incremental softmax for this
            group: inv_nrm (ACT), scores *= inv_nrm (DVE), exp (ACT), partial
            per-r sums accumulated into s1 (DVE)."""
            gs = slice(g * TPG, (g + 1) * TPG)
            fd_g = fd_pool.tile([128, TPG, W // 2], F16)
            nc.vector.tensor_add(
                fd_g[:], sq_g[:, :, 0 : W // 2], sq_g[:, :, W // 2 : W]
            )
            fd2_g = fd2_pool.tile([128, TPG, W // 4], F16)
            nc.vector.tensor_add(
                fd2_g[:], fd_g[:, :, 0 : W // 4], fd_g[:, :, W // 4 : W // 2]
            )
            nc.vector.reduce_sum(ss[:, gs], fd2_g[:], axis=mybir.AxisListType.X)
            lss_g = smalls.tile([128, TPG], F32, tag="lssg")
            nc.scalar.activation(lss_g[:], ss[:, gs], AF.Ln)
            inm_g = smalls.tile([128, TPG], F32, tag="inmg")
            nc.scalar.activation(inm_g[:], lss_g[:], AF.Exp, scale=-0.5)
            nc.vector.tensor_mul(
                scores[:, gs, :],
                scores[:, gs, :],
                inm_g[:].unsqueeze(2).broadcast_to([128, TPG, R]),
            )
            nc.scalar.activation(scores[:, gs, :], scores[:, gs, :], AF.Exp)
            s1g = smalls.tile([128, R], F32, tag="s1g")
            nc.vector.reduce_sum(
                s1g[:],
                scores[:, gs, :].transpose([0, 2, 1]),
                axis=mybir.AxisListType.X,
            )
            if first:
                nc.vector.tensor_copy(s1[:], s1g[:])
            else:
                nc.vector.tensor_add(s1[:], s1[:], s1g[:])

        for b in range(BLOC):
            # ---- read-vector prep: rv' = rv * strength / ||rv|| ----
            rv_t = smalls.tile([128, R], F32)
            nc.sync.dma_start(rv_t[:], rv[b])
            rs_t = smalls.tile([1, R], F32)
            nc.sync.dma_start(rs_t[:], rs[b : b + 1, :])
            rs_h = smalls.tile([1, R], F16)
            nc.scalar.copy(rs_h[:], rs_t[:])

            rv2 = smalls.tile([128, R], F16)
            nc.vector.tensor_mul(rv2[:], rv_t[:], rv_t[:])
            nv2_ps = rtps_pool.tile([128, R], F32, tag="prep")
            nc.tensor.matmul(nv2_ps[:], ones_h[:], rv2[:], start=True, stop=True)
            lnv = smalls.tile([128, R], F32)
            nc.scalar.activation(lnv[:], nv2_ps[:], AF.Ln)
            inv_nv = smalls.tile([128, R], F32)
            nc.scalar.activation(inv_nv[:], lnv[:], AF.Exp, scale=-0.5)
            rsb_ps = rtps_pool.tile([128, R], F32, tag="prep")
            nc.tensor.matmul(
                rsb_ps[:], ones_h[0:1, :], rs_h[:], start=True, stop=True
            )
            factor = smalls.tile([128, R], F32)
            nc.vector.tensor_mul(factor[:], rsb_ps[:], inv_nv[:])
            rvp = smalls.tile([128, R], F32, tag="rvp")
            nc.vector.tensor_mul(rvp[:], rv_t[:], factor[:])
            rvp_h = smalls.tile([128, R], F16, tag="rvph")
            nc.scalar.copy(rvp_h[:], rvp[:])

            scores = score_pool.tile([128, T, R], F32)
            ss = ss_pool.tile([128, T], F32)
            s1 = smalls.tile([128, R], F32, tag="s1")
            pending = []  # (sq_g, g) awaiting deferred norm emission

            for g in range(NG):
                mem_g = in_pool.tile([128, TPG, W], F16)
                src = mem[b, g * TPG * 128 : (g + 1) * TPG * 128, :].rearrange(
                    "(p t) w -> p t w", p=128
                )
                nc.gpsimd.dma_start(mem_g[:], src)  # casting DMA f32->f16

                sq_g = sq_pool.tile([128, TPG, W], F16)
                se = SQUARE_ENGINES[sq_i % len(SQUARE_ENGINES)]
                sq_i += 1
                nsp = 4 if se == "g" else 2
                sp = TPG // nsp
                for k in range(nsp):
                    sl = (slice(None), slice(k * sp, (k + 1) * sp), slice(None))
                    if se == "g":
                        nc.gpsimd.tensor_mul(sq_g[sl], mem_g[sl], mem_g[sl])
                    elif se == "v":
                        nc.vector.tensor_mul(sq_g[sl], mem_g[sl], mem_g[sl])
                    else:
                        nc.scalar.square(sq_g[sl], mem_g[sl])
                pending.append((sq_g, g))

                scps = scps_pool.tile([128, TPG * R], F32)
                for q in range(NCH):  # 8-tile chunks (1024 n)
                    mt_ps = mtps_pool.tile([128, CH * 128], F16)
                    for j in range(CH):
                        tt = q * CH + j
                        nc.tensor.transpose(
                            mt_ps[:, j * 128 : (j + 1) * 128],
                            mem_g[:, tt, :],
                            id_h[:],
                        )
                    mt_sb = mt_pool.tile([128, CH * 128], F16)
                    de = MEMT_DRAIN[drain_i % len(MEMT_DRAIN)]
                    drain_i += 1
                    if de == "s":
                        nc.scalar.copy(mt_sb[:], mt_ps[:])
                    else:
                        nc.vector.tensor_copy(mt_sb[:], mt_ps[:])

                    for j in range(CH):
                        tt = q * CH + j
                        nc.tensor.matmul(
                            scps[:, tt * R : (tt + 1) * R],
                            mt_sb[:, j * 128 : (j + 1) * 128],
                            rvp_h[:],
                            start=True,
                            stop=True,
                        )
                nc.scalar.copy(
                    scores[:, g * TPG : (g + 1) * TPG, :],
                    scps[:].rearrange("p (t r) -> p t r", r=R),
                )

                # deferred norm chain (chase-proofing: producers long done)
                if len(pending) > NORM_DEFER:
                    psq, pg = pending.pop(0)
                    emit_norm(ss, psq, pg, scores, s1, first=(pg == 0))

            for psq, pg in pending:
                emit_norm(ss, psq, pg, scores, s1, first=(pg == 0))
            pending = []

            # ---- softmax finalization (exp'd scores + s1 already done) ----
            s1_h = smalls.tile([128, R], F16)
            nc.scalar.copy(s1_h[:], s1[:])
            tot_ps = rtps_pool.tile([128, R], F32, tag="prep")
            nc.tensor.matmul(tot_ps[:], ones_h[:], s1_h[:], start=True, stop=True)
            inv_tot = smalls.tile([128, R], F32)
            nc.vector.reciprocal(inv_tot[:], tot_ps[:])
            H = T // 2
            for h in range(2):
                hs = slice(h * H, (h + 1) * H)
                nc.vector.tensor_mul(
                    scores[:, hs, :],
                    scores[:, hs, :],
                    inv_tot[:].unsqueeze(1).broadcast_to([128, H, R]),
                )
                nc.scalar.dma_start(out[b, :, hs, :], scores[:, hs, :])

    nc.compile()
    return nc


_program = None
last_results = None


def _get_program():
    global _program
    if _program is None:
        _program = build_program()
    return _program


def kernel(memory, read_strengths, read_vectors):
    memory = np.asarray(memory, dtype=np.float32)
    read_strengths = np.asarray(read_strengths, dtype=np.float32)
    read_vectors = np.asarray(read_vectors, dtype=np.float32)

    nc = _get_program()
    identity = np.eye(128, dtype=np.float32)
    ones_m = np.ones((128, 128), dtype=np.float32)
    in_maps = []
    for c in range(NCORES):
        sl = slice(c * BLOC, (c + 1) * BLOC)
        in_maps.append(
            {
                "memory": np.ascontiguousarray(memory[sl]),
                "read_vectors": np.ascontiguousarray(read_vectors[sl]),
                "read_strengths": np.ascontiguousarray(read_strengths[sl]),
                "identity": identity,
                "ones": ones_m,
            }
        )

    global last_results
    last_results = run_bass_kernel_spmd(nc, in_maps, list(range(NCORES)))
    res = last_results.results
    outs = []
    for c in range(NCORES):
        o = np.asarray(res[c]["out"])  # (BLOC, 128, T=NG*TPG, R); n = g*4096 + p*32 + t
        o = o.reshape(BLOC, 128, NG, TPG, R).transpose(0, 2, 1, 3, 4)
        outs.append(o.reshape(BLOC, N, R))
    return np.concatenate(outs, axis=0)


# revision 12
# speedup vs baseline: 1.1281x; 1.1281x over previous
"""Content-based addressing read (DNC-style) for Trainium2.

Computes softmax_n( strengths[r] * cos_sim(memory[b,n,:], read_vectors[b,:,r]) )
for B=16, N=32768, W=128, R=8, sharded batch-parallel across 8 NeuronCores
(2 batches per core).

v7: 16-bit datapath + decoupled scheduling.
  - gpsimd casting DMA (f32 HBM -> f16 SBUF). The DMA issue instructions are
    emitted with a 4-group LOOKAHEAD so they never queue behind multi-us
    gpsimd squares (in-order queue) -- the input stream stays continuous.
  - PE: 8 transposes per 1024-wide f16 PSUM tile + 8 f16 sim matmuls
    (memT chunk stationary, rv' moving, f32 PSUM accumulate).
  - norms: square (gpsimd/DVE/ACT rotation, split into slices) -> two
    fold-halves adds (DVE f16 2x) -> w-reduce (DVE, f32 out). Norm chains are
    emitted NORM_DEFER groups late (chase-proofing: the in-order DVE queue
    would otherwise chase slow producers element-wise) and BEFORE the next
    group's drains to avoid queue pile-ups.
  - batch 0's softmax tail is emitted inside batch 1's second group so it
    overlaps batch 1's DMA/compute instead of stalling it.
Softmax math stays fp32; no max subtraction (|scores| <= ~1); the reference's
+1e-8 is a provable fp32 no-op (normalizer ~128); 1/sqrt via exp(-0.5*ln).

Output is stored in DRAM as (b, p, t, r) with n = g*4096 + p*32 + t; the host
re-transposes the 16MB result to (b, n, r).
"""

import sys

for _p in ("/opt/trn_rl_repo",):
    if _p not in sys.path:
        sys.path.insert(0, _p)

from contextlib import ExitStack

import numpy as np

import concourse.bass as bass
import concourse.bacc as bacc
import concourse.tile as tile
from concourse import mybir
from concourse import bass_isa
from concourse.bass_utils import run_bass_kernel_spmd

F32 = mybir.dt.float32
F16 = mybir.dt.float16
AF = mybir.ActivationFunctionType

B, N, W, R = 16, 32768, 128, 8
NCORES = 8
BLOC = B // NCORES          # batches per core
T = N // 128                # 256 n-tiles of 128 per batch
NG = 8                      # DMA groups per batch
TPG = T // NG               # 32 tiles per group (4096 n, 2MB)
CH = 8                      # tiles per PSUM transpose chunk (1024 cols)
NCH = TPG // CH             # chunks per group
NSTEP = BLOC * NG           # 16 flat steps

# ---- tuning knobs ----
SQUARE_ENGINES = "gvgs"     # g=GpSimd, v=DVE, s=ScalarE
MEMT_DRAIN = "vs"           # rotation for memT PSUM->SBUF drains
NORM_DEFER = 2              # steps of deferral for fold/reduce emission
DMA_AHEAD = 4               # DMA issue lookahead (must be < IN_BUFS - 2)
IN_BUFS = 9


def build_program():
    nc = bacc.Bacc("TRN2", target_bir_lowering=False, debug=False, num_devices=NCORES)

    mem = nc.dram_tensor("memory", [BLOC, N, W], F32, kind="ExternalInput").ap()
    rv = nc.dram_tensor("read_vectors", [BLOC, W, R], F32, kind="ExternalInput").ap()
    rs = nc.dram_tensor("read_strengths", [BLOC, R], F32, kind="ExternalInput").ap()
    ident = nc.dram_tensor("identity", [128, 128], F32, kind="ExternalInput").ap()
    ones = nc.dram_tensor("ones", [128, 128], F32, kind="ExternalInput").ap()
    out = nc.dram_tensor("out", [BLOC, 128, T, R], F32, kind="ExternalOutput").ap()

    with ExitStack() as ctx:
        tc = ctx.enter_context(tile.TileContext(nc))

        const_pool = ctx.enter_context(tc.tile_pool(name="const", bufs=1))
        id_t = const_pool.tile([128, 128], F32)
        nc.sync.dma_start(id_t[:], ident)
        ones_t = const_pool.tile([128, 128], F32)
        nc.sync.dma_start(ones_t[:], ones)
        id_h = const_pool.tile([128, 128], F16)
        nc.scalar.copy(id_h[:], id_t[:])
        ones_h = const_pool.tile([128, 128], F16)
        nc.scalar.copy(ones_h[:], ones_t[:])

        in_pool = ctx.enter_context(tc.tile_pool(name="mem_in", bufs=IN_BUFS))
        sq_pool = ctx.enter_context(tc.tile_pool(name="sq", bufs=NORM_DEFER + 2))
        fd_pool = ctx.enter_context(tc.tile_pool(name="fd", bufs=2))
        fd2_pool = ctx.enter_context(tc.tile_pool(name="fd2", bufs=2))
        mtps_pool = ctx.enter_context(tc.tile_pool(name="mtps", bufs=4, space="PSUM"))
        mt_pool = ctx.enter_context(tc.tile_pool(name="mt", bufs=6))
        scps_pool = ctx.enter_context(tc.tile_pool(name="scps", bufs=2, space="PSUM"))
        rtps_pool = ctx.enter_context(tc.tile_pool(name="rtps", bufs=1, space="PSUM"))
        smalls = ctx.enter_context(tc.tile_pool(name="smalls", bufs=2))
        score_pool = ctx.enter_context(tc.tile_pool(name="scores", bufs=2))
        ss_pool = ctx.enter_context(tc.tile_pool(name="ss", bufs=2))

        state = {"drain_i": 0, "sq_i": 0}

        # per-batch state
        scores_t = [None] * BLOC
        ss_t = [None] * BLOC
        rvp_t = [None] * BLOC
        mem_tiles = {}  # flat step -> mem_g tile
        sq_tiles = {}   # flat step -> sq tile

        def issue_dma(step):
            b, g = divmod(step, NG)
            mem_g = in_pool.tile([128, TPG, W], F16)
            src = mem[b, g * TPG * 128 : (g + 1) * TPG * 128, :].rearrange(
                "(p t) w -> p t w", p=128
            )
            nc.gpsimd.dma_start(mem_g[:], src)  # casting DMA f32->f16
            mem_tiles[step] = mem_g

        def rv_prep(b):
            rv_t = smalls.tile([128, R], F32)
            nc.sync.dma_start(rv_t[:], rv[b])
            rs_t = smalls.tile([1, R], F32)
            nc.sync.dma_start(rs_t[:], rs[b : b + 1, :])
            rs_h = smalls.tile([1, R], F16)
            nc.scalar.copy(rs_h[:], rs_t[:])

            rv2 = smalls.tile([128, R], F16)
            nc.vector.tensor_mul(rv2[:], rv_t[:], rv_t[:])
            nv2_ps = rtps_pool.tile([128, R], F32, tag="prep")
            nc.tensor.matmul(nv2_ps[:], ones_h[:], rv2[:], start=True, stop=True)
            lnv = smalls.tile([128, R], F32)
            nc.scalar.activation(lnv[:], nv2_ps[:], AF.Ln)
            inv_nv = smalls.tile([128, R], F32)
            nc.scalar.activation(inv_nv[:], lnv[:], AF.Exp, scale=-0.5)
            rsb_ps = rtps_pool.tile([128, R], F32, tag="prep")
            nc.tensor.matmul(
                rsb_ps[:], ones_h[0:1, :], rs_h[:], start=True, stop=True
            )
            factor = smalls.tile([128, R], F32)
            nc.vector.tensor_mul(factor[:], rsb_ps[:], inv_nv[:])
            rvp = smalls.tile([128, R], F32, tag="rvp")
            nc.vector.tensor_mul(rvp[:], rv_t[:], factor[:])
            rvp_h = smalls.tile([128, R], F16, tag="rvph")
            nc.scalar.copy(rvp_h[:], rvp[:])
            rvp_t[b] = rvp_h

        def emit_norm(step):
            """fold halves twice (f16 2x) then reduce (f32 out) on DVE."""
            b, g = divmod(step, NG)
            sq_g = sq_tiles.pop(step)
            ss = ss_t[b]
            fd_g = fd_pool.tile([128, TPG, W // 2], F16)
            nc.vector.tensor_add(
                fd_g[:], sq_g[:, :, 0 : W // 2], sq_g[:, :, W // 2 : W]
            )
            fd2_g = fd2_pool.tile([128, TPG, W // 4], F16)
            nc.vector.tensor_add(
                fd2_g[:], fd_g[:, :, 0 : W // 4], fd_g[:, :, W // 4 : W // 2]
            )
            nc.vector.reduce_sum(
                ss[:, g * TPG : (g + 1) * TPG],
                fd2_g[:],
                axis=mybir.AxisListType.X,
            )

        def softmax_tail(b):
            scores = scores_t[b]
            ss = ss_t[b]
            lss = smalls.tile([128, T], F32, tag="lsst")
            nc.scalar.activation(lss[:], ss[:], AF.Ln)
            inv_nrm = smalls.tile([128, T], F32, tag="invnrm")
            nc.scalar.activation(inv_nrm[:], lss[:], AF.Exp, scale=-0.5)

            nc.vector.tensor_mul(
                scores[:],
                scores[:],
                inv_nrm[:].unsqueeze(2).broadcast_to([128, T, R]),
            )
            nc.scalar.activation(scores[:], scores[:], AF.Exp)

            s1 = smalls.tile([128, R], F32, tag="s1")
            nc.vector.reduce_sum(
                s1[:], scores[:].transpose([0, 2, 1]), axis=mybir.AxisListType.X
            )
            s1_h = smalls.tile([128, R], F16)
            nc.scalar.copy(s1_h[:], s1[:])
            tot_ps = rtps_pool.tile([128, R], F32, tag="prep")
            nc.tensor.matmul(tot_ps[:], ones_h[:], s1_h[:], start=True, stop=True)
            inv_tot = smalls.tile([128, R], F32, tag="invtot")
            nc.vector.reciprocal(inv_tot[:], tot_ps[:])
            H = T // 2
            for h in range(2):
                hs = slice(h * H, (h + 1) * H)
                nc.vector.tensor_mul(
                    scores[:, hs, :],
                    scores[:, hs, :],
                    inv_tot[:].unsqueeze(1).broadcast_to([128, H, R]),
                )
                nc.scalar.dma_start(out[b, :, hs, :], scores[:, hs, :])

        # prologue: fill DMA pipeline, prep batch 0
        for s0 in range(DMA_AHEAD):
            issue_dma(s0)
        rv_prep(0)

        for step in range(NSTEP):
            b, g = divmod(step, NG)
            if g == 0:
                scores_new = score_pool.tile([128, T, R], F32, tag="scores")
                ss_new = ss_pool.tile([128, T], F32, tag="ss")
                scores_t[b] = scores_new
                ss_t[b] = ss_new
            if step + DMA_AHEAD < NSTEP:
                issue_dma(step + DMA_AHEAD)
            # batch b+1 rv-prep midway through batch b
            if g == NG - 3 and b + 1 < BLOC:
                rv_prep(b + 1)

            mem_g = mem_tiles.pop(step)
            sq_g = sq_pool.tile([128, TPG, W], F16)
            se = SQUARE_ENGINES[state["sq_i"] % len(SQUARE_ENGINES)]
            state["sq_i"] += 1
            nsp = 4 if se == "g" else 2
            sp = TPG // nsp
            for k in range(nsp):
                sl = (slice(None), slice(k * sp, (k + 1) * sp), slice(None))
                if se == "g":
                    nc.gpsimd.tensor_mul(sq_g[sl], mem_g[sl], mem_g[sl])
                elif se == "v":
                    nc.vector.tensor_mul(sq_g[sl], mem_g[sl], mem_g[sl])
                else:
                    nc.scalar.square(sq_g[sl], mem_g[sl])
            sq_tiles[step] = sq_g

            # deferred norm chain BEFORE this group's drains (spreads V load)
            if step >= NORM_DEFER:
                emit_norm(step - NORM_DEFER)
            # batch b-1 softmax overlapped into batch b's second group
            if g == 1 and b > 0:
                softmax_tail(b - 1)

            scores = scores_t[b]
            scps = scps_pool.tile([128, TPG * R], F32)
            for q in range(NCH):  # 8-tile chunks (1024 n)
                mt_ps = mtps_pool.tile([128, CH * 128], F16)
                for j in range(CH):
                    tt = q * CH + j
                    nc.tensor.transpose(
                        mt_ps[:, j * 128 : (j + 1) * 128],
                        mem_g[:, tt, :],
                        id_h[:],
                    )
                mt_sb = mt_pool.tile([128, CH * 128], F16)
                de = MEMT_DRAIN[state["drain_i"] % len(MEMT_DRAIN)]
                state["drain_i"] += 1
                if de == "s":
                    nc.scalar.copy(mt_sb[:], mt_ps[:])
                else:
                    nc.vector.tensor_copy(mt_sb[:], mt_ps[:])

                for j in range(CH):
                    tt = q * CH + j
                    nc.tensor.matmul(
                        scps[:, tt * R : (tt + 1) * R],
                        mt_sb[:, j * 128 : (j + 1) * 128],
                        rvp_t[b][:],
                        start=True,
                        stop=True,
                    )
            nc.scalar.copy(
                scores[:, g * TPG : (g + 1) * TPG, :],
                scps[:].rearrange("p (t r) -> p t r", r=R),
            )

        # epilogue: flush remaining norms + last batch softmax
        for step in range(NSTEP - NORM_DEFER, NSTEP):
            emit_norm(step)
        softmax_tail(BLOC - 1)

    nc.compile()
    return nc


_program = None
last_results = None


def _get_program():
    global _program
    if _program is None:
        _program = build_program()
    return _program


def kernel(memory, read_strengths, read_vectors):
    memory = np.asarray(memory, dtype=np.float32)
    read_strengths = np.asarray(read_strengths, dtype=np.float32)
    read_vectors = np.asarray(read_vectors, dtype=np.float32)

    nc = _get_program()
    identity = np.eye(128, dtype=np.float32)
    ones_m = np.ones((128, 128), dtype=np.float32)
    in_maps = []
    for c in range(NCORES):
        sl = slice(c * BLOC, (c + 1) * BLOC)
        in_maps.append(
            {
                "memory": np.ascontiguousarray(memory[sl]),
                "read_vectors": np.ascontiguousarray(read_vectors[sl]),
                "read_strengths": np.ascontiguousarray(read_strengths[sl]),
                "identity": identity,
                "ones": ones_m,
            }
        )

    global last_results
    last_results = run_bass_kernel_spmd(nc, in_maps, list(range(NCORES)))
    res = last_results.results
    outs = []
    for c in range(NCORES):
        o = np.asarray(res[c]["out"])  # (BLOC, 128, T=NG*TPG, R); n = g*4096 + p*32 + t
        o = o.reshape(BLOC, 128, NG, TPG, R).transpose(0, 2, 1, 3, 4)
        outs.append(o.reshape(BLOC, N, R))
    return np.concatenate(outs, axis=0)
